# revision 16
# baseline (speedup 1.0000x reference)
"""Trainium2 Bass kernel for a device-aware top-1 MoE layer.

Strategy (expert parallelism over 8 NeuronCores):
  - Host: gate + top-1 routing, gather each expert's tokens. Experts are
    paired big-with-small onto cores: slot0 capacity C0 = max big count,
    slot1 capacity C1 = max small count.
  - All weights fp8 E3M4 (w1 x2^8, w2 x2^9); x pre-scaled by 2^-8 in bf16
    so stage-1 PSUM is unscaled and the 2^-9 dequant rides the stage-2
    epilogue. Host-measured end-to-end rel err 1.85e-2 (< 2e-2 gate).
  - Per slot: x pieces (2 d-chunks) interleaved with w1 groups on the sync
    queue so stage 1 tracks the HBM stream. Stage 1 pass 0 runs 7 PSUM
    chains (h0..6) d-outer behind the stream; warmup matmuls (8th PSUM
    bank) are interleaved at w1 group boundaries so the PE never idles
    and the HAM clock governor keeps the PE at full frequency. Pass 1
    (h7..15) is h-outer on resident weights, retiring chains eagerly.
  - Stage 2: two w2 groups; group 0 h-outer (consumed as the DMA lands),
    group 1 d-outer so each of the 8 accumulators retires as soon as its
    h=15 matmul issues -- epilogues and y output DMAs (2-d-chunk pieces
    on the gpsimd SWDGE queue) overlap the remaining matmuls instead of
    trailing the kernel.
"""

import numpy as np
import ml_dtypes

D = 1024
H = 2048
E = 16
NCORES = 8
P = 128
DB = D // P    # 8 d-chunks
HB = H // P    # 16 h-chunks
W2C = 8        # h-chunks per w2 DMA group (2 groups)
S1 = 8         # w1 scale exponent: w1q = w1 * 2^S1, x' = x * 2^-S1
S2 = 9         # w2 scale exponent: w2q = w2 * 2^S2, y = acc * 2^-S2 + b2
NWARM0 = 29    # initial PE warmup matmuls (HAM clock ramp + DMA lead-in)
WFILL = {1: 6, 3: 8, 5: 6}  # warm matmuls after pass-0 d-iter (stream fill)
NP0 = 7        # pass-0 PSUM chains (bank 8 is the warmup accumulator)
# w1 DMA groups (d0, ndc). Every dma_start costs ~600ns of descriptor-gen
# on the issuing sequencer, so the second-processed slot (resident well
# before use) takes 2 big groups while the first keeps 4 for finer stream
# tracking. Slot 1 (fewer lead-in bytes) is processed first.
W1PLANS = (
    [(0, 2), (2, 2), (4, 2), (6, 2)],
    [(0, 4), (4, 4)],
)
# Big slot first: its longer (C0-col) matmuls keep the PE duty cycle high
# during the DMA lead-in, which the HAM clock governor rewards with a much
# faster frequency ramp (processing the small slot first measured slower).
SLOT_ORDER = (0, 1)
W1DMAPS = []   # per slot: d -> (group index, local offset)
for _plan in W1PLANS:
    _m = []
    for _gi, (_d0, _n) in enumerate(_plan):
        for _l in range(_n):
            _m.append((_gi, _l))
    W1DMAPS.append(_m)

_program_cache = {}


def _build_program(C0, C1):
    """Per-core Bass/Tile program; slot capacities C0 (big), C1 (small)."""
    import concourse.tile as tile
    from concourse import bacc, mybir

    assert C0 <= 512 and C1 <= 512
    f32 = mybir.dt.float32
    bf16 = mybir.dt.bfloat16
    f8 = mybir.dt.float8e3
    AF = mybir.ActivationFunctionType
    ALU = mybir.AluOpType
    CS = (C0, C1)

    nc = bacc.Bacc(
        "TRN2", target_bir_lowering=False, debug=False, num_devices=NCORES
    )
    xT = nc.dram_tensor("xT", [P, DB * (C0 + C1)], bf16, kind="ExternalInput").ap()
    w1q = nc.dram_tensor("w1q", [2, P, DB * H], f8, kind="ExternalInput").ap()
    w2q = nc.dram_tensor("w2q", [2, P, HB * D], f8, kind="ExternalInput").ap()
    b1s = nc.dram_tensor("b1s", [2, P, HB], f32, kind="ExternalInput").ap()
    b2s = nc.dram_tensor("b2s", [2, P, DB], f32, kind="ExternalInput").ap()
    yT = nc.dram_tensor("yT", [P, DB * (C0 + C1)], bf16, kind="ExternalOutput").ap()

    with tile.TileContext(nc) as tc:
        with (
            tc.tile_pool(name="xp", bufs=2) as xp,
            tc.tile_pool(name="w1p", bufs=8) as w1p,
            tc.tile_pool(name="w2p", bufs=4) as w2p,
            tc.tile_pool(name="hp", bufs=32) as hp,
            tc.tile_pool(name="bp", bufs=4) as bp,
            tc.tile_pool(name="yp", bufs=8) as yp,
            tc.tile_pool(name="wm", bufs=1) as wm,
            tc.tile_pool(name="ps", bufs=8, space="PSUM") as ps,
        ):
            # Warmup stationary/moving tile + its dedicated PSUM bank
            # (allocated first so it sits at ring slot 0).
            wt = wm.tile([P, 2 * P], bf16, tag="warm")
            wps = ps.tile([P, P], f32, tag="acc", name="warmps")

            def warm(n):
                for _ in range(n):
                    nc.tensor.matmul(
                        wps[:], lhsT=wt[:, 0:P], rhs=wt[:, P:2 * P],
                        start=True, stop=True,
                    )

            b1ts, b2ts = [], []
            for s in range(2):
                b1t = bp.tile([P, HB], f32, tag="b1")
                nc.gpsimd.dma_start(b1t[:], b1s[s])
                b1ts.append(b1t)
                b2t = bp.tile([P, DB], f32, tag="b2")
                nc.gpsimd.dma_start(b2t[:], b2s[s])
                b2ts.append(b2t)

            nc.vector.memset(wt[:], 1.0)

            def epi1(i, out_t, acc_t, bias_col):
                """relu(acc + b1), alternating engines."""
                if i % 2 == 0:
                    nc.scalar.activation(out_t, acc_t, AF.Relu, bias=bias_col)
                else:
                    nc.vector.tensor_scalar(
                        out_t, acc_t, bias_col, 0.0, ALU.add, ALU.max
                    )

            def epi2(i, out_t, acc_t, bias_col):
                """acc * 2^-S2 + b2, alternating engines."""
                if i % 2 == 0:
                    nc.scalar.activation(
                        out_t, acc_t, AF.Identity, bias=bias_col,
                        scale=float(2.0 ** -S2),
                    )
                else:
                    nc.vector.tensor_scalar(
                        out_t, acc_t, float(2.0 ** -S2), bias_col,
                        ALU.mult, ALU.add,
                    )

            ydmas = []
            for si, s in enumerate(SLOT_ORDER):
                Cs = CS[s]
                first = si == 0
                xoff = 0 if s == 0 else DB * CS[0]
                # Sync issue order, first slot: x_a, g0, g1, x_b, g2, g3 --
                # x_b is first needed at pass-0 d=4, after g1's consumers.
                xt = xp.tile([P, DB * Cs], bf16, tag="xT", name=f"x_{s}")

                def xload(dlo, dhi):
                    nc.sync.dma_start(
                        xt[:, dlo * Cs:dhi * Cs],
                        xT[:, xoff + dlo * Cs:xoff + dhi * Cs],
                    )

                w1ts = []

                def w1load(gi):
                    d0, ndc = W1PLANS[s][gi]
                    w1t = w1p.tile([P, ndc * H], f8, tag="w1",
                                   name=f"w1_{s}_{gi}")
                    nc.sync.dma_start(
                        w1t[:], w1q[s][:, d0 * H:(d0 + ndc) * H]
                    )
                    w1ts.append(w1t)

                if first:
                    xload(0, 4)
                    w1load(0)
                    w1load(1)
                    xload(4, DB)
                    w1load(2)
                    w1load(3)
                else:
                    xload(0, DB)
                    w1load(0)
                    w1load(1)

                def w1col(d, h):
                    gi, l = W1DMAPS[s][d]
                    return w1ts[gi][:, l * H + h * P:l * H + h * P + P]

                # ---- stage 1: hT = relu(w1q.T @ xT + b1) ----
                hts = [None] * HB
                if first:
                    warm(NWARM0)
                # pass 0 (h 0..NP0-1): d-outer so the PE tracks the w1
                # stream; warm fills at group boundaries keep it busy.
                accs1 = [
                    ps.tile([P, Cs], f32, tag="acc", name=f"a1_{s}_{i}")
                    for i in range(NP0)
                ]
                for d in range(DB):
                    xd = xt[:, d * Cs:(d + 1) * Cs]
                    for h in range(NP0):
                        nc.tensor.matmul(
                            accs1[h][:], lhsT=w1col(d, h), rhs=xd,
                            start=(d == 0), stop=(d == DB - 1),
                        )
                    if first and d in WFILL:
                        warm(WFILL[d])
                for h in range(NP0):
                    ht = hp.tile([P, Cs], bf16, tag="hT", name=f"h_{s}_{h}")
                    epi1(h, ht[:], accs1[h][:], b1ts[s][:, h:h + 1])
                    hts[h] = ht
                # pass 1 (h NP0..15): weights resident -> h-outer, retire
                # each psum immediately so epilogues pipeline.
                for h in range(NP0, HB):
                    acc = ps.tile([P, Cs], f32, tag="acc", name=f"a1b_{s}_{h}")
                    for d in range(DB):
                        nc.tensor.matmul(
                            acc[:], lhsT=w1col(d, h),
                            rhs=xt[:, d * Cs:(d + 1) * Cs],
                            start=(d == 0), stop=(d == DB - 1),
                        )
                    ht = hp.tile([P, Cs], bf16, tag="hT", name=f"h_{s}_{h}")
                    epi1(h, ht[:], acc[:], b1ts[s][:, h:h + 1])
                    hts[h] = ht

                # ---- stage 2: yT = (w2q.T @ hT) * 2^-S2 + b2 ----
                accs2 = [
                    ps.tile([P, Cs], f32, tag="acc", name=f"a2_{s}_{d}")
                    for d in range(DB)
                ]
                w2ts = []
                for g in range(HB // W2C):
                    w2t = w2p.tile([P, W2C * D], f8, tag="w2",
                                   name=f"w2_{s}_{g}")
                    nc.sync.dma_start(
                        w2t[:], w2q[s][:, g * W2C * D:(g + 1) * W2C * D]
                    )
                    w2ts.append(w2t)
                # group 0 (h 0..7): h-outer, consumed as the DMA lands.
                for hh in range(W2C):
                    for d in range(DB):
                        nc.tensor.matmul(
                            accs2[d][:],
                            lhsT=w2ts[0][:, hh * D + d * P:hh * D + d * P + P],
                            rhs=hts[hh][:],
                            start=(hh == 0), stop=False,
                        )
                # group 1 (h 8..15): d-outer so each accumulator retires as
                # soon as its h=15 matmul issues; epilogues + y DMAs overlap
                # the remaining matmuls. y streams per 2-d-chunk piece.
                yts = [
                    yp.tile([P, 2 * Cs], bf16, tag="yt", name=f"y_{s}_{j}")
                    for j in range(DB // 2)
                ]
                for d in range(DB):
                    for hh in range(W2C):
                        nc.tensor.matmul(
                            accs2[d][:],
                            lhsT=w2ts[1][:, hh * D + d * P:hh * D + d * P + P],
                            rhs=hts[W2C + hh][:],
                            start=False, stop=(hh == W2C - 1),
                        )
                    yt = yts[d // 2]
                    epi2(d, yt[:, (d % 2) * Cs:(d % 2 + 1) * Cs],
                         accs2[d][:], b2ts[s][:, d:d + 1])
                    if d % 2 == 1:
                        ydmas.append(
                            (yT[:, xoff + (d - 1) * Cs:
                                 xoff + (d + 1) * Cs], yt[:])
                        )

            # y DMAs issue on the fast sync HWDGE, but are emitted last so
            # their descriptor-gen sits behind every load in the sync
            # queue (no head-of-line blocking of the second slot's weights).
            for dst, src in ydmas:
                nc.sync.dma_start(dst, src)

    nc.compile()
    return nc


def _ceil4(n):
    return max(32, (int(n) + 3) // 4 * 4)


def kernel(x, gate_w, gate_b, w1, b1, w2, b2, _trace=False):
    from concourse.bass_utils import run_bass_kernel_spmd

    x = np.asarray(x, dtype=np.float32)
    B, S, d_in = x.shape
    T = B * S
    xf = x.reshape(T, d_in)

    # --- routing (host side: the dispatch/sharding step) ---
    logits = xf @ np.asarray(gate_w, dtype=np.float32) + np.asarray(
        gate_b, dtype=np.float32
    )
    top1 = np.argmax(logits, axis=-1)
    idxs = [np.nonzero(top1 == e)[0] for e in range(E)]
    counts = np.array([len(i) for i in idxs])
    order = np.argsort(-counts, kind="stable")
    slot0_ids = order[:NCORES]              # big experts, one per core
    slot1_ids = order[NCORES:][::-1]        # paired smallest-with-biggest
    C0 = _ceil4(counts[slot0_ids].max())
    C1 = _ceil4(counts[slot1_ids].max())
    C0, C1 = min(C0, 512), min(C1, 512)
    assert counts[slot0_ids].max() <= C0 and counts[slot1_ids].max() <= C1, (
        "expert capacity overflow"
    )

    if (C0, C1) not in _program_cache:
        _program_cache[(C0, C1)] = _build_program(C0, C1)
    nc = _program_cache[(C0, C1)]

    bf16 = ml_dtypes.bfloat16
    e3m4 = ml_dtypes.float8_e3m4
    w1 = np.asarray(w1, dtype=np.float32)
    w2 = np.asarray(w2, dtype=np.float32)
    b1 = np.asarray(b1, dtype=np.float32)
    b2 = np.asarray(b2, dtype=np.float32)

    # Quantize all experts at once, in the [P, ...]-major DMA layouts.
    # w1 row d = o*P + p  ->  w1qh[e, p, o*H:(o+1)*H]
    w1qh = np.ascontiguousarray(
        (w1 * 2.0 ** S1).astype(e3m4).reshape(E, DB, P, H).transpose(0, 2, 1, 3)
    ).reshape(E, P, DB * H)
    w2qh = np.ascontiguousarray(
        (w2 * 2.0 ** S2).astype(e3m4).reshape(E, HB, P, D).transpose(0, 2, 1, 3)
    ).reshape(E, P, HB * D)
    b1h = b1.reshape(E, HB, P).transpose(0, 2, 1)   # [E, P, HB]
    b2h = b2.reshape(E, DB, P).transpose(0, 2, 1)   # [E, P, DB]

    xs = (xf * 2.0 ** -S1).astype(bf16)

    in_maps = []
    pair = [(int(slot0_ids[c]), int(slot1_ids[c])) for c in range(NCORES)]
    for core in range(NCORES):
        xT = np.zeros((P, DB * (C0 + C1)), dtype=bf16)
        off = 0
        for s, Cs in ((0, C0), (1, C1)):
            e = pair[core][s]
            idx = idxs[e]
            if len(idx):
                # [len, D] -> [P, DB, len]
                blk = xs[idx].T.reshape(DB, P, len(idx)).transpose(1, 0, 2)
                xv = xT[:, off:off + DB * Cs].reshape(P, DB, Cs)
                xv[:, :, :len(idx)] = blk
            off += DB * Cs
        es = [pair[core][0], pair[core][1]]
        in_maps.append({
            "xT": xT,
            "w1q": w1qh[es],
            "w2q": w2qh[es],
            "b1s": np.ascontiguousarray(b1h[es]),
            "b2s": np.ascontiguousarray(b2h[es]),
        })

    res = run_bass_kernel_spmd(
        nc, in_maps, core_ids=list(range(NCORES)), trace=_trace
    )

    out = np.zeros((T, D), dtype=np.float32)
    for core in range(NCORES):
        yT_out = res.results[core]["yT"]
        off = 0
        for s, Cs in ((0, C0), (1, C1)):
            e = pair[core][s]
            idx = idxs[e]
            if len(idx):
                blk = yT_out[:, off:off + DB * Cs].reshape(P, DB, Cs)
                # [P, DB, Cs] -> [D, Cs] with row d = o*P + p
                yD = blk.transpose(1, 0, 2).reshape(D, Cs)
                out[idx] = yD[:, :len(idx)].T.astype(np.float32)
            off += DB * Cs
    if _trace:
        kernel.last_result = res
    return out.reshape(B, S, D)


# revision 18
# speedup vs baseline: 1.0258x; 1.0258x over previous
"""Trainium2 Bass kernel for a device-aware top-1 MoE layer.

Strategy (expert parallelism over 8 NeuronCores):
  - Host: gate + top-1 routing, gather each expert's tokens. Experts are
    paired big-with-small onto cores: slot0 capacity C0 = max big count,
    slot1 capacity C1 = max small count.
  - All weights fp8 E3M4 (w1 x2^8, w2 x2^9); x pre-scaled by 2^-8 in bf16
    so stage-1 PSUM is unscaled and the 2^-9 dequant rides the stage-2
    epilogue. Host-measured end-to-end rel err 1.85e-2 (< 2e-2 gate).
  - Per slot: x pieces (2 d-chunks) interleaved with w1 groups on the sync
    queue so stage 1 tracks the HBM stream. Stage 1 pass 0 runs 7 PSUM
    chains (h0..6) d-outer behind the stream; warmup matmuls (8th PSUM
    bank) are interleaved at w1 group boundaries so the PE never idles
    and the HAM clock governor keeps the PE at full frequency. Pass 1
    (h7..15) is h-outer on resident weights, retiring chains eagerly.
  - Stage 2: two w2 groups; group 0 h-outer (consumed as the DMA lands),
    group 1 d-outer so each of the 8 accumulators retires as soon as its
    h=15 matmul issues -- epilogues and y output DMAs (2-d-chunk pieces
    on the gpsimd SWDGE queue) overlap the remaining matmuls instead of
    trailing the kernel.
"""

import numpy as np
import ml_dtypes

D = 1024
H = 2048
E = 16
NCORES = 8
P = 128
DB = D // P    # 8 d-chunks
HB = H // P    # 16 h-chunks
W2C = 8        # h-chunks per w2 DMA group (2 groups)
S1 = 8         # w1 scale exponent: w1q = w1 * 2^S1, x' = x * 2^-S1
S2 = 9         # w2 scale exponent: w2q = w2 * 2^S2, y = acc * 2^-S2 + b2
NWARM0 = 29    # initial PE warmup matmuls (HAM clock ramp + DMA lead-in)
WFILL = {1: 6, 3: 8, 5: 6}  # warm matmuls after pass-0 d-iter (stream fill)
NP0 = 7        # pass-0 PSUM chains (bank 8 is the warmup accumulator)
# w1 DMA groups (d0, ndc). Every dma_start costs ~600ns of descriptor-gen
# on the issuing sequencer, so the second-processed slot (resident well
# before use) takes 2 big groups while the first keeps 4 for finer stream
# tracking. Slot 1 (fewer lead-in bytes) is processed first.
W1PLANS = (
    [(0, 2), (2, 2), (4, 2), (6, 2)],
    [(0, 4), (4, 4)],
)
# Big slot first: its longer (C0-col) matmuls keep the PE duty cycle high
# during the DMA lead-in, which the HAM clock governor rewards with a much
# faster frequency ramp (processing the small slot first measured slower).
SLOT_ORDER = (0, 1)
W1DMAPS = []   # per slot: d -> (group index, local offset)
for _plan in W1PLANS:
    _m = []
    for _gi, (_d0, _n) in enumerate(_plan):
        for _l in range(_n):
            _m.append((_gi, _l))
    W1DMAPS.append(_m)

_program_cache = {}


def _build_program(C0, C1):
    """Per-core Bass/Tile program; slot capacities C0 (big), C1 (small)."""
    import concourse.tile as tile
    from concourse import bacc, mybir

    assert C0 <= 512 and C1 <= 512
    f32 = mybir.dt.float32
    bf16 = mybir.dt.bfloat16
    f8 = mybir.dt.float8e3
    AF = mybir.ActivationFunctionType
    ALU = mybir.AluOpType
    CS = (C0, C1)

    nc = bacc.Bacc(
        "TRN2", target_bir_lowering=False, debug=False, num_devices=NCORES
    )
    xT = nc.dram_tensor("xT", [P, DB * (C0 + C1)], bf16, kind="ExternalInput").ap()
    w1q = nc.dram_tensor("w1q", [2, P, DB * H], f8, kind="ExternalInput").ap()
    w2q = nc.dram_tensor("w2q", [2, P, HB * D], f8, kind="ExternalInput").ap()
    b1s = nc.dram_tensor("b1s", [2, P, HB], f32, kind="ExternalInput").ap()
    b2s = nc.dram_tensor("b2s", [2, P, DB], f32, kind="ExternalInput").ap()
    yT = nc.dram_tensor("yT", [P, DB * (C0 + C1)], bf16, kind="ExternalOutput").ap()

    with tile.TileContext(nc) as tc:
        with (
            tc.tile_pool(name="xp", bufs=2) as xp,
            tc.tile_pool(name="w1p", bufs=8) as w1p,
            tc.tile_pool(name="w2p", bufs=4) as w2p,
            tc.tile_pool(name="hp", bufs=32) as hp,
            tc.tile_pool(name="bp", bufs=4) as bp,
            tc.tile_pool(name="yp", bufs=8) as yp,
            tc.tile_pool(name="wm", bufs=1) as wm,
            tc.tile_pool(name="ps", bufs=8, space="PSUM") as ps,
        ):
            # Warmup stationary/moving tile + its dedicated PSUM bank
            # (allocated first so it sits at ring slot 0).
            wt = wm.tile([P, 2 * P], bf16, tag="warm")
            wps = ps.tile([P, P], f32, tag="acc", name="warmps")

            def warm(n):
                for _ in range(n):
                    nc.tensor.matmul(
                        wps[:], lhsT=wt[:, 0:P], rhs=wt[:, P:2 * P],
                        start=True, stop=True,
                    )

            b1ts, b2ts = [], []
            for s in range(2):
                b1t = bp.tile([P, HB], f32, tag="b1")
                nc.gpsimd.dma_start(b1t[:], b1s[s])
                b1ts.append(b1t)
                b2t = bp.tile([P, DB], f32, tag="b2")
                nc.gpsimd.dma_start(b2t[:], b2s[s])
                b2ts.append(b2t)

            nc.vector.memset(wt[:], 1.0)

            def epi1(i, out_t, acc_t, bias_col):
                """relu(acc + b1), alternating engines."""
                if i % 2 == 0:
                    nc.scalar.activation(out_t, acc_t, AF.Relu, bias=bias_col)
                else:
                    nc.vector.tensor_scalar(
                        out_t, acc_t, bias_col, 0.0, ALU.add, ALU.max
                    )

            def epi2(i, out_t, acc_t, bias_col):
                """acc * 2^-S2 + b2, alternating engines."""
                if i % 2 == 0:
                    nc.scalar.activation(
                        out_t, acc_t, AF.Identity, bias=bias_col,
                        scale=float(2.0 ** -S2),
                    )
                else:
                    nc.vector.tensor_scalar(
                        out_t, acc_t, float(2.0 ** -S2), bias_col,
                        ALU.mult, ALU.add,
                    )

            ydmas = []
            for si, s in enumerate(SLOT_ORDER):
                Cs = CS[s]
                first = si == 0
                xoff = 0 if s == 0 else DB * CS[0]
                # Sync issue order, first slot: x_a, g0, g1, x_b, g2, g3 --
                # x_b is first needed at pass-0 d=4, after g1's consumers.
                xt = xp.tile([P, DB * Cs], bf16, tag="xT", name=f"x_{s}")

                def xload(dlo, dhi):
                    nc.sync.dma_start(
                        xt[:, dlo * Cs:dhi * Cs],
                        xT[:, xoff + dlo * Cs:xoff + dhi * Cs],
                    )

                w1ts = []

                def w1load(gi):
                    d0, ndc = W1PLANS[s][gi]
                    w1t = w1p.tile([P, ndc * H], f8, tag="w1",
                                   name=f"w1_{s}_{gi}")
                    nc.sync.dma_start(
                        w1t[:], w1q[s][:, d0 * H:(d0 + ndc) * H]
                    )
                    w1ts.append(w1t)

                if first:
                    xload(0, 4)
                    w1load(0)
                    w1load(1)
                    xload(4, DB)
                    w1load(2)
                    w1load(3)
                else:
                    xload(0, DB)
                    w1load(0)
                    w1load(1)

                def w1col(d, h):
                    gi, l = W1DMAPS[s][d]
                    return w1ts[gi][:, l * H + h * P:l * H + h * P + P]

                # ---- stage 1: hT = relu(w1q.T @ xT + b1) ----
                hts = [None] * HB
                if first:
                    warm(NWARM0)
                # pass 0 (h 0..NP0-1): d-outer so the PE tracks the w1
                # stream; warm fills at group boundaries keep it busy.
                accs1 = [
                    ps.tile([P, Cs], f32, tag="acc", name=f"a1_{s}_{i}")
                    for i in range(NP0)
                ]
                for d in range(DB):
                    xd = xt[:, d * Cs:(d + 1) * Cs]
                    for h in range(NP0):
                        nc.tensor.matmul(
                            accs1[h][:], lhsT=w1col(d, h), rhs=xd,
                            start=(d == 0), stop=(d == DB - 1),
                        )
                    if first and d in WFILL:
                        warm(WFILL[d])
                for h in range(NP0):
                    ht = hp.tile([P, Cs], bf16, tag="hT", name=f"h_{s}_{h}")
                    epi1(h, ht[:], accs1[h][:], b1ts[s][:, h:h + 1])
                    hts[h] = ht
                # pass 1 (h NP0..15): weights resident -> h-outer, retire
                # each psum immediately so epilogues pipeline.
                for h in range(NP0, HB):
                    acc = ps.tile([P, Cs], f32, tag="acc", name=f"a1b_{s}_{h}")
                    for d in range(DB):
                        nc.tensor.matmul(
                            acc[:], lhsT=w1col(d, h),
                            rhs=xt[:, d * Cs:(d + 1) * Cs],
                            start=(d == 0), stop=(d == DB - 1),
                        )
                    ht = hp.tile([P, Cs], bf16, tag="hT", name=f"h_{s}_{h}")
                    epi1(h, ht[:], acc[:], b1ts[s][:, h:h + 1])
                    hts[h] = ht

                # ---- stage 2: yT = (w2q.T @ hT) * 2^-S2 + b2 ----
                accs2 = [
                    ps.tile([P, Cs], f32, tag="acc", name=f"a2_{s}_{d}")
                    for d in range(DB)
                ]
                w2ts = []
                for g in range(HB // W2C):
                    w2t = w2p.tile([P, W2C * D], f8, tag="w2",
                                   name=f"w2_{s}_{g}")
                    nc.sync.dma_start(
                        w2t[:], w2q[s][:, g * W2C * D:(g + 1) * W2C * D]
                    )
                    w2ts.append(w2t)
                # group 0 (h 0..7): h-outer, consumed as the DMA lands.
                for hh in range(W2C):
                    for d in range(DB):
                        nc.tensor.matmul(
                            accs2[d][:],
                            lhsT=w2ts[0][:, hh * D + d * P:hh * D + d * P + P],
                            rhs=hts[hh][:],
                            start=(hh == 0), stop=False,
                        )
                # group 1 (h 8..15): d-outer so each accumulator retires as
                # soon as its h=15 matmul issues; epilogues + y DMAs overlap
                # the remaining matmuls. y streams per 2-d-chunk piece.
                # y piece width: 2 d-chunks while fully overlapped (first
                # slot), 4 d-chunks for the second slot so the kernel tail
                # waits on one issue instead of a serialized chain of four.
                ypc = 2 if first else 4
                yts = [
                    yp.tile([P, ypc * Cs], bf16, tag="yt", name=f"y_{s}_{j}")
                    for j in range(DB // ypc)
                ]
                for d in range(DB):
                    for hh in range(W2C):
                        nc.tensor.matmul(
                            accs2[d][:],
                            lhsT=w2ts[1][:, hh * D + d * P:hh * D + d * P + P],
                            rhs=hts[W2C + hh][:],
                            start=False, stop=(hh == W2C - 1),
                        )
                    yt = yts[d // ypc]
                    epi2(d, yt[:, (d % ypc) * Cs:(d % ypc + 1) * Cs],
                         accs2[d][:], b2ts[s][:, d:d + 1])
                    if d % ypc == ypc - 1:
                        ydmas.append(
                            (yT[:, xoff + (d - ypc + 1) * Cs:
                                 xoff + (d + 1) * Cs], yt[:])
                        )

            # y DMAs issue on the fast sync HWDGE, but are emitted last so
            # their descriptor-gen sits behind every load in the sync
            # queue (no head-of-line blocking of the second slot's weights).
            for dst, src in ydmas:
                nc.sync.dma_start(dst, src)

    nc.compile()
    return nc


def _ceil4(n):
    return max(32, (int(n) + 3) // 4 * 4)


def kernel(x, gate_w, gate_b, w1, b1, w2, b2, _trace=False):
    from concourse.bass_utils import run_bass_kernel_spmd

    x = np.asarray(x, dtype=np.float32)
    B, S, d_in = x.shape
    T = B * S
    xf = x.reshape(T, d_in)

    # --- routing (host side: the dispatch/sharding step) ---
    logits = xf @ np.asarray(gate_w, dtype=np.float32) + np.asarray(
        gate_b, dtype=np.float32
    )
    top1 = np.argmax(logits, axis=-1)
    idxs = [np.nonzero(top1 == e)[0] for e in range(E)]
    counts = np.array([len(i) for i in idxs])
    order = np.argsort(-counts, kind="stable")
    slot0_ids = order[:NCORES]              # big experts, one per core
    slot1_ids = order[NCORES:][::-1]        # paired smallest-with-biggest
    C0 = _ceil4(counts[slot0_ids].max())
    C1 = _ceil4(counts[slot1_ids].max())
    C0, C1 = min(C0, 512), min(C1, 512)
    assert counts[slot0_ids].max() <= C0 and counts[slot1_ids].max() <= C1, (
        "expert capacity overflow"
    )

    if (C0, C1) not in _program_cache:
        _program_cache[(C0, C1)] = _build_program(C0, C1)
    nc = _program_cache[(C0, C1)]

    bf16 = ml_dtypes.bfloat16
    e3m4 = ml_dtypes.float8_e3m4
    w1 = np.asarray(w1, dtype=np.float32)
    w2 = np.asarray(w2, dtype=np.float32)
    b1 = np.asarray(b1, dtype=np.float32)
    b2 = np.asarray(b2, dtype=np.float32)

    # Quantize all experts at once, in the [P, ...]-major DMA layouts.
    # w1 row d = o*P + p  ->  w1qh[e, p, o*H:(o+1)*H]
    w1qh = np.ascontiguousarray(
        (w1 * 2.0 ** S1).astype(e3m4).reshape(E, DB, P, H).transpose(0, 2, 1, 3)
    ).reshape(E, P, DB * H)
    w2qh = np.ascontiguousarray(
        (w2 * 2.0 ** S2).astype(e3m4).reshape(E, HB, P, D).transpose(0, 2, 1, 3)
    ).reshape(E, P, HB * D)
    b1h = b1.reshape(E, HB, P).transpose(0, 2, 1)   # [E, P, HB]
    b2h = b2.reshape(E, DB, P).transpose(0, 2, 1)   # [E, P, DB]

    xs = (xf * 2.0 ** -S1).astype(bf16)

    in_maps = []
    pair = [(int(slot0_ids[c]), int(slot1_ids[c])) for c in range(NCORES)]
    for core in range(NCORES):
        xT = np.zeros((P, DB * (C0 + C1)), dtype=bf16)
        off = 0
        for s, Cs in ((0, C0), (1, C1)):
            e = pair[core][s]
            idx = idxs[e]
            if len(idx):
                # [len, D] -> [P, DB, len]
                blk = xs[idx].T.reshape(DB, P, len(idx)).transpose(1, 0, 2)
                xv = xT[:, off:off + DB * Cs].reshape(P, DB, Cs)
                xv[:, :, :len(idx)] = blk
            off += DB * Cs
        es = [pair[core][0], pair[core][1]]
        in_maps.append({
            "xT": xT,
            "w1q": w1qh[es],
            "w2q": w2qh[es],
            "b1s": np.ascontiguousarray(b1h[es]),
            "b2s": np.ascontiguousarray(b2h[es]),
        })

    res = run_bass_kernel_spmd(
        nc, in_maps, core_ids=list(range(NCORES)), trace=_trace
    )

    out = np.zeros((T, D), dtype=np.float32)
    for core in range(NCORES):
        yT_out = res.results[core]["yT"]
        off = 0
        for s, Cs in ((0, C0), (1, C1)):
            e = pair[core][s]
            idx = idxs[e]
            if len(idx):
                blk = yT_out[:, off:off + DB * Cs].reshape(P, DB, Cs)
                # [P, DB, Cs] -> [D, Cs] with row d = o*P + p
                yD = blk.transpose(1, 0, 2).reshape(D, Cs)
                out[idx] = yD[:, :len(idx)].T.astype(np.float32)
            off += DB * Cs
    if _trace:
        kernel.last_result = res
    return out.reshape(B, S, D)
